# revision 24
# baseline (speedup 1.0000x reference)
"""Trainium2 Bass kernel for nn_DistributionSampler.

Reference computation (per (batch b, head h)):
  scores  = (K[b,h] @ q[b,h,0,:]) / 8                       # [S]
  attn    = softmax-ish: exp(scores - max) * mask, + eps/S, / (sum + eps)
  top-k   = top-1024 of log(attn) + gumbel(key 42)          # fixed noise
  out     = scatter True at (idx + 1), True at 0, drop OOB  # -> bool [S]

Key identities used by this kernel:
  * The per-head normalizer (max subtraction and sum division) only shifts
    log(attn) by a per-head constant, so the top-k SET is unchanged if we
    use keys = log(exp(dot/8) * mask + eps/S) + gumbel  directly (no max,
    no sum). exp(dot/8) <= e^6-ish, no overflow.
  * The scatter is a shift-by-one: out[j] = (j == 0) | topk_mask[j - 1].
  * Top-k membership == keys > t where t is found by bisection on the
    count of keys above the threshold (exact once the bisection interval
    no longer contains any key; verified bit-exact vs the reference).

Sharding: pure data parallel, core c <- batch c (B=8, 8 cores), all 12
heads per core. No collectives.
"""

import numpy as np

B, H, S, D = 8, 12, 8192, 64
NS = 1024                       # num sampled
EPS = 1e-6
EPS_S = float(np.float32(EPS / S))
N_CORES = 8
P = 128                         # SBUF partitions
A = S // P                      # 64 s-rows per partition in score layout
OCT = 8                         # partitions per head in bisect layout
SB = S // OCT                   # 1024 keys per bisect partition
LO0, HI0 = 1.5, 3.5             # bisection bracket (holds w/ huge margin)
N_ITERS = 22

_cache = {}


def _gumbel_np():
    if "g" not in _cache:
        import jax
        import jax.numpy as jnp

        g = jax.random.gumbel(jax.random.key(42), (B, H, S), dtype=jnp.float32)
        _cache["g"] = np.asarray(g)
    return _cache["g"]


def _build_nc():
    import concourse.bacc as bacc
    import concourse.mybir as mybir
    from concourse.tile import TileContext

    f32 = mybir.dt.float32
    u8 = mybir.dt.uint8
    Alu = mybir.AluOpType
    Act = mybir.ActivationFunctionType

    nc = bacc.Bacc("TRN2", target_bir_lowering=False)

    k_h = nc.dram_tensor("k", [H, S, D], f32, kind="ExternalInput")
    q_h = nc.dram_tensor("q", [H, D], f32, kind="ExternalInput")
    tm_h = nc.dram_tensor("tm", [S], f32, kind="ExternalInput")
    g_h = nc.dram_tensor("g", [H, S], f32, kind="ExternalInput")
    m96_h = nc.dram_tensor("m96", [H * OCT, H * OCT], f32, kind="ExternalInput")
    out_h = nc.dram_tensor("out", [H, S], u8, kind="ExternalOutput")

    NP96 = H * OCT  # 96

    with (
        TileContext(nc) as tc,
        tc.tile_pool(name="kpool", bufs=4) as kpool,
        tc.tile_pool(name="ppool", bufs=2) as ppool,
        tc.tile_pool(name="spool", bufs=3) as spool,
        tc.tile_pool(name="persist", bufs=1) as persist,
        tc.tile_pool(name="psum", bufs=1, space="PSUM") as psum,
    ):
        # ---- persistent small tensors -------------------------------
        mask_sb = persist.tile([P, A], f32, name="mask_sb")
        nc.scalar.dma_start(out=mask_sb, in_=tm_h[:].rearrange("(p a) -> p a", p=P))

        m96_sb = persist.tile([NP96, NP96], f32, name="m96_sb")
        nc.scalar.dma_start(out=m96_sb, in_=m96_h[:, :])

        keysB = persist.tile([NP96, SB], f32, name="keysB")
        e_all = persist.tile([P, H * A], f32, name="e_all")
        gum_all = persist.tile([P, H * A], f32, name="gum_all")
        # gum_all[p, h*A + a] = g[h, 64p + a]
        nc.scalar.dma_start(
            out=gum_all,
            in_=g_h[:, :].rearrange("h (p a) -> p h a", p=P),
        )

        lo_t = persist.tile([NP96, 1], f32, name="lo_t")
        hi_t = persist.tile([NP96, 1], f32, name="hi_t")
        nc.vector.memset(lo_t, LO0)
        nc.vector.memset(hi_t, HI0)

        ones_u8 = persist.tile([H, 1], u8, name="ones_u8")
        nc.vector.memset(ones_u8, 1)
        nc.scalar.dma_start(out=out_h[:, 0:1], in_=ones_u8)

        epss_bias = persist.tile([P, 1], f32, name="epss_bias")
        nc.vector.memset(epss_bias, EPS_S)

        # q broadcast to all 128 partitions via a K=1 matmul:
        # qrep_ps[p, (h d)] = q[h, d] for every partition p.
        ones_row = persist.tile([1, P], f32, name="ones_row")
        nc.vector.memset(ones_row, 1.0)
        q_all = persist.tile([1, H * D], f32, name="q_all")
        nc.scalar.dma_start(out=q_all, in_=q_h[:, :])

        # [128, 1024] = 2 PSUM banks; two matmuls so neither output AP
        # crosses a bank boundary (N<=512 fp32 per matmul).
        qrep_ps = psum.tile([P, 1024], f32, name="qrep_ps")
        nc.tensor.matmul(
            qrep_ps[:, 0:512], ones_row, q_all[:, 0:512], start=True, stop=True
        )
        nc.tensor.matmul(
            qrep_ps[:, 512 : H * D],
            ones_row,
            q_all[:, 512 : H * D],
            start=True,
            stop=True,
        )

        # ---- per-head score/keys phase ------------------------------
        for h in range(H):
            k_sb = kpool.tile([P, A * D], f32, name="k_sb", tag="k_sb")
            nc.sync.dma_start(
                out=k_sb, in_=k_h[h, :, :].rearrange("(p a) d -> p (a d)", p=P)
            )

            prod = ppool.tile([P, A * D], f32, name="prod", tag="prod")
            nc.vector.tensor_mul(
                prod.rearrange("p (a d) -> p a d", d=D),
                k_sb.rearrange("p (a d) -> p a d", d=D),
                qrep_ps[:, h * D : (h + 1) * D]
                .rearrange("p (x d) -> p x d", x=1)
                .to_broadcast([P, A, D]),
            )

            scores = spool.tile([P, A], f32, name="scores", tag="scores")
            nc.vector.tensor_reduce(
                scores,
                prod.rearrange("p (a d) -> p a d", d=D),
                axis=mybir.AxisListType.X,
                op=Alu.add,
            )

            nc.scalar.activation(
                e_all[:, h * A : (h + 1) * A], scores, Act.Exp, bias=0.0, scale=0.125
            )

        # ---- batched mask/log/gumbel over all heads ------------------
        em_all = persist.tile([P, H * A], f32, name="em_all")
        nc.vector.tensor_mul(
            em_all.rearrange("p (h a) -> p h a", a=A),
            e_all.rearrange("p (h a) -> p h a", a=A),
            mask_sb.rearrange("p (x a) -> p x a", x=1).to_broadcast([P, H, A]),
        )
        lg_all = persist.tile([P, H * A], f32, name="lg_all")
        nc.scalar.activation(lg_all, em_all, Act.Ln, bias=epss_bias, scale=1.0)
        keys_all = persist.tile([P, H * A], f32, name="keys_all")
        nc.vector.tensor_add(keys_all, lg_all, gum_all)
        for h in range(H):
            # [128, 64] column block -> [8, 1024] (same s-order stream)
            nc.scalar.dma_start(
                out=keysB[h * OCT : (h + 1) * OCT, :],
                in_=keys_all[:, h * A : (h + 1) * A],
            )

        # ---- bisection ----------------------------------------------
        trash = psum.tile([NP96, SB], f32, name="trash")
        cntall96p = psum.tile([NP96, 1], f32, name="cntall96p", bufs=2)

        for it in range(N_ITERS):
            negsum = spool.tile([NP96, 1], f32, name="negsum", tag="negsum")
            # negsum = -(lo + hi), replicated per octant partition
            nc.vector.scalar_tensor_tensor(
                out=negsum,
                in0=lo_t,
                scalar=-1.0,
                in1=hi_t,
                op0=Alu.mult,
                op1=Alu.subtract,
            )

            cnt96 = spool.tile([NP96, 1], f32, name="cnt96", tag="cnt96")
            # sign(2*keys - (lo+hi)) summed per partition
            nc.scalar.activation(
                trash,
                keysB,
                Act.Sign,
                bias=negsum,
                scale=2.0,
                accum_out=cnt96,
            )
            # per-head totals, redistributed to every octant partition:
            # cntall96p = blockdiag(ones 8x8).T @ cnt96
            nc.tensor.matmul(cntall96p, m96_sb, cnt96, start=True, stop=True)

            pred = spool.tile([NP96, 1], mybir.dt.uint32, name="pred", tag="pred")
            nc.vector.tensor_scalar(pred, cntall96p, -6144.0, None, Alu.is_ge)
            predn = spool.tile(
                [NP96, 1], mybir.dt.uint32, name="predn", tag="predn"
            )
            nc.vector.tensor_scalar(predn, cntall96p, -6144.0, None, Alu.is_lt)
            mid = spool.tile([NP96, 1], f32, name="mid", tag="mid")
            nc.vector.tensor_scalar(mid, negsum, -0.5, None, Alu.mult)

            nc.vector.copy_predicated(lo_t, pred, mid)
            nc.vector.copy_predicated(hi_t, predn, mid)

        # ---- final mask + output ------------------------------------
        maskB = persist.tile([NP96, SB], u8, name="maskB")
        nc.vector.tensor_scalar(maskB, keysB, lo_t, None, Alu.is_gt)

        # shifted mask
        for h in range(H):
            eng = nc.sync if h % 2 == 0 else nc.scalar
            eng.dma_start(
                out=out_h[h, 1 : 1 + (OCT - 1) * SB].rearrange(
                    "(o x) -> o x", o=OCT - 1
                ),
                in_=maskB[h * OCT : h * OCT + OCT - 1, :],
            )
            eng.dma_start(
                out=out_h[h : h + 1, 1 + (OCT - 1) * SB : S],
                in_=maskB[h * OCT + OCT - 1 : h * OCT + OCT, 0 : SB - 1],
            )

    nc.compile()
    return nc


def _get_nc():
    if "nc" not in _cache:
        _cache["nc"] = _build_nc()
    return _cache["nc"]


def _const_inputs():
    if "consts" not in _cache:
        m96 = np.zeros((H * OCT, H * OCT), dtype=np.float32)
        for h in range(H):
            m96[h * OCT : (h + 1) * OCT, h * OCT : (h + 1) * OCT] = 1.0
        _cache["consts"] = {"m96": m96}
    return _cache["consts"]


def kernel(q, k, v, token_mask):
    from concourse.bass_utils import run_bass_kernel_spmd

    q = np.ascontiguousarray(np.asarray(q, dtype=np.float32))
    k = np.ascontiguousarray(np.asarray(k, dtype=np.float32))
    token_mask = np.ascontiguousarray(np.asarray(token_mask, dtype=np.float32))

    g = _gumbel_np()
    consts = _const_inputs()
    q0 = q[:, :, 0, :]  # [B, H, D]

    nc = _get_nc()
    in_maps = [
        {
            "k": np.ascontiguousarray(k[c]),
            "q": np.ascontiguousarray(q0[c]),
            "tm": token_mask[c],
            "g": np.ascontiguousarray(g[c]),
            "m96": consts["m96"],
        }
        for c in range(N_CORES)
    ]
    res = run_bass_kernel_spmd(nc, in_maps, core_ids=list(range(N_CORES)))
    _cache["last_res"] = res
    out = np.stack([res.results[c]["out"] for c in range(N_CORES)], axis=0)
    return out.astype(bool)


# revision 25
# speedup vs baseline: 1.0695x; 1.0695x over previous
"""Trainium2 Bass kernel for nn_DistributionSampler.

Reference computation (per (batch b, head h)):
  scores  = (K[b,h] @ q[b,h,0,:]) / 8                       # [S]
  attn    = softmax-ish: exp(scores - max) * mask, + eps/S, / (sum + eps)
  top-k   = top-1024 of log(attn) + gumbel(key 42)          # fixed noise
  out     = scatter True at (idx + 1), True at 0, drop OOB  # -> bool [S]

Key identities used by this kernel:
  * The per-head normalizer (max subtraction and sum division) only shifts
    log(attn) by a per-head constant, so the top-k SET is unchanged if we
    use keys = log(exp(dot/8) * mask + eps/S) + gumbel  directly (no max,
    no sum). exp(dot/8) <= e^6-ish, no overflow.
  * The scatter is a shift-by-one: out[j] = (j == 0) | topk_mask[j - 1].
  * Top-k membership == keys > t where t is found by bisection on the
    count of keys above the threshold (exact once the bisection interval
    no longer contains any key; verified bit-exact vs the reference).

Sharding: pure data parallel, core c <- batch c (B=8, 8 cores), all 12
heads per core. No collectives.
"""

import numpy as np

B, H, S, D = 8, 12, 8192, 64
NS = 1024                       # num sampled
EPS = 1e-6
EPS_S = float(np.float32(EPS / S))
N_CORES = 8
P = 128                         # SBUF partitions
A = S // P                      # 64 s-rows per partition in score layout
OCT = 8                         # partitions per head in bisect layout
SB = S // OCT                   # 1024 keys per bisect partition
LO0, HI0 = 1.5, 3.5             # bisection bracket (holds w/ huge margin)
N_ITERS = 22

_cache = {}


def _gumbel_np():
    if "g" not in _cache:
        import jax
        import jax.numpy as jnp

        g = jax.random.gumbel(jax.random.key(42), (B, H, S), dtype=jnp.float32)
        _cache["g"] = np.asarray(g)
    return _cache["g"]


def _build_nc():
    import concourse.bacc as bacc
    import concourse.mybir as mybir
    from concourse.tile import TileContext

    f32 = mybir.dt.float32
    u8 = mybir.dt.uint8
    Alu = mybir.AluOpType
    Act = mybir.ActivationFunctionType

    nc = bacc.Bacc("TRN2", target_bir_lowering=False)

    k_h = nc.dram_tensor("k", [H, S, D], f32, kind="ExternalInput")
    q_h = nc.dram_tensor("q", [H, D], f32, kind="ExternalInput")
    tm_h = nc.dram_tensor("tm", [S], f32, kind="ExternalInput")
    g_h = nc.dram_tensor("g", [H, S], f32, kind="ExternalInput")
    m96_h = nc.dram_tensor("m96", [H * OCT, H * OCT], f32, kind="ExternalInput")
    out_h = nc.dram_tensor("out", [H, S], u8, kind="ExternalOutput")

    NP96 = H * OCT  # 96

    with (
        TileContext(nc) as tc,
        tc.tile_pool(name="kpool", bufs=4) as kpool,
        tc.tile_pool(name="ppool", bufs=2) as ppool,
        tc.tile_pool(name="spool", bufs=3) as spool,
        tc.tile_pool(name="persist", bufs=1) as persist,
        tc.tile_pool(name="psum", bufs=1, space="PSUM") as psum,
    ):
        # ---- persistent small tensors -------------------------------
        mask_sb = persist.tile([P, A], f32, name="mask_sb")
        nc.sync.dma_start(out=mask_sb, in_=tm_h[:].rearrange("(p a) -> p a", p=P))

        m96_sb = persist.tile([NP96, NP96], f32, name="m96_sb")
        nc.sync.dma_start(out=m96_sb, in_=m96_h[:, :])

        keysB = persist.tile([NP96, SB], f32, name="keysB")
        e_all = persist.tile([P, H * A], f32, name="e_all")
        gum_all = persist.tile([P, H * A], f32, name="gum_all")
        # gum_all[p, h*A + a] = g[h, 64p + a]
        nc.sync.dma_start(
            out=gum_all,
            in_=g_h[:, :].rearrange("h (p a) -> p h a", p=P),
        )

        lo_t = persist.tile([NP96, 1], f32, name="lo_t")
        hi_t = persist.tile([NP96, 1], f32, name="hi_t")
        nc.vector.memset(lo_t, LO0)
        nc.vector.memset(hi_t, HI0)

        ones_u8 = persist.tile([H, 1], u8, name="ones_u8")
        nc.vector.memset(ones_u8, 1)
        nc.sync.dma_start(out=out_h[:, 0:1], in_=ones_u8)

        epss_bias = persist.tile([P, 1], f32, name="epss_bias")
        nc.vector.memset(epss_bias, EPS_S)

        # q broadcast to all 128 partitions via a K=1 matmul:
        # qrep_ps[p, (h d)] = q[h, d] for every partition p.
        ones_row = persist.tile([1, P], f32, name="ones_row")
        nc.vector.memset(ones_row, 1.0)
        q_all = persist.tile([1, H * D], f32, name="q_all")
        nc.sync.dma_start(out=q_all, in_=q_h[:, :])

        # [128, 1024] = 2 PSUM banks; two matmuls so neither output AP
        # crosses a bank boundary (N<=512 fp32 per matmul).
        qrep_ps = psum.tile([P, 1024], f32, name="qrep_ps")
        nc.tensor.matmul(
            qrep_ps[:, 0:512], ones_row, q_all[:, 0:512], start=True, stop=True
        )
        nc.tensor.matmul(
            qrep_ps[:, 512 : H * D],
            ones_row,
            q_all[:, 512 : H * D],
            start=True,
            stop=True,
        )

        # ---- per-head score/keys phase ------------------------------
        for h in range(H):
            k_sb = kpool.tile([P, A * D], f32, name="k_sb", tag="k_sb")
            nc.sync.dma_start(
                out=k_sb, in_=k_h[h, :, :].rearrange("(p a) d -> p (a d)", p=P)
            )

            prod = ppool.tile([P, A * D], f32, name="prod", tag="prod")
            nc.vector.tensor_mul(
                prod.rearrange("p (a d) -> p a d", d=D),
                k_sb.rearrange("p (a d) -> p a d", d=D),
                qrep_ps[:, h * D : (h + 1) * D]
                .rearrange("p (x d) -> p x d", x=1)
                .to_broadcast([P, A, D]),
            )

            scores = spool.tile([P, A], f32, name="scores", tag="scores")
            nc.vector.tensor_reduce(
                scores,
                prod.rearrange("p (a d) -> p a d", d=D),
                axis=mybir.AxisListType.X,
                op=Alu.add,
            )

            nc.scalar.activation(
                e_all[:, h * A : (h + 1) * A], scores, Act.Exp, bias=0.0, scale=0.125
            )

        # ---- batched mask/log/gumbel over all heads ------------------
        em_all = persist.tile([P, H * A], f32, name="em_all")
        nc.vector.tensor_mul(
            em_all.rearrange("p (h a) -> p h a", a=A),
            e_all.rearrange("p (h a) -> p h a", a=A),
            mask_sb.rearrange("p (x a) -> p x a", x=1).to_broadcast([P, H, A]),
        )
        lg_all = persist.tile([P, H * A], f32, name="lg_all")
        nc.scalar.activation(lg_all, em_all, Act.Ln, bias=epss_bias, scale=1.0)
        keys_all = persist.tile([P, H * A], f32, name="keys_all")
        nc.vector.tensor_add(keys_all, lg_all, gum_all)
        for h in range(H):
            # [128, 64] column block -> [8, 1024] (same s-order stream)
            eng = nc.sync if h % 2 == 0 else nc.scalar
            eng.dma_start(
                out=keysB[h * OCT : (h + 1) * OCT, :],
                in_=keys_all[:, h * A : (h + 1) * A],
            )

        # ---- bisection ----------------------------------------------
        trash = psum.tile([NP96, SB], f32, name="trash")
        cntall96p = psum.tile([NP96, 1], f32, name="cntall96p", bufs=2)

        for it in range(N_ITERS):
            negsum = spool.tile([NP96, 1], f32, name="negsum", tag="negsum")
            # negsum = -(lo + hi), replicated per octant partition
            nc.vector.scalar_tensor_tensor(
                out=negsum,
                in0=lo_t,
                scalar=-1.0,
                in1=hi_t,
                op0=Alu.mult,
                op1=Alu.subtract,
            )

            cnt96 = spool.tile([NP96, 1], f32, name="cnt96", tag="cnt96")
            # sign(2*keys - (lo+hi)) summed per partition
            nc.scalar.activation(
                trash,
                keysB,
                Act.Sign,
                bias=negsum,
                scale=2.0,
                accum_out=cnt96,
            )
            # per-head totals, redistributed to every octant partition:
            # cntall96p = blockdiag(ones 8x8).T @ cnt96
            nc.tensor.matmul(cntall96p, m96_sb, cnt96, start=True, stop=True)

            pred = spool.tile([NP96, 1], mybir.dt.uint32, name="pred", tag="pred")
            nc.vector.tensor_scalar(pred, cntall96p, -6144.0, None, Alu.is_ge)
            predn = spool.tile(
                [NP96, 1], mybir.dt.uint32, name="predn", tag="predn"
            )
            nc.vector.tensor_scalar(predn, cntall96p, -6144.0, None, Alu.is_lt)
            mid = spool.tile([NP96, 1], f32, name="mid", tag="mid")
            nc.vector.tensor_scalar(mid, negsum, -0.5, None, Alu.mult)

            nc.vector.copy_predicated(lo_t, pred, mid)
            nc.vector.copy_predicated(hi_t, predn, mid)

        # ---- final mask + output ------------------------------------
        maskB = persist.tile([NP96, SB], u8, name="maskB")
        nc.vector.tensor_scalar(maskB, keysB, lo_t, None, Alu.is_gt)

        # shifted mask
        for h in range(H):
            eng = nc.sync if h % 2 == 0 else nc.scalar
            eng.dma_start(
                out=out_h[h, 1 : 1 + (OCT - 1) * SB].rearrange(
                    "(o x) -> o x", o=OCT - 1
                ),
                in_=maskB[h * OCT : h * OCT + OCT - 1, :],
            )
            eng.dma_start(
                out=out_h[h : h + 1, 1 + (OCT - 1) * SB : S],
                in_=maskB[h * OCT + OCT - 1 : h * OCT + OCT, 0 : SB - 1],
            )

    nc.compile()
    return nc


def _get_nc():
    if "nc" not in _cache:
        _cache["nc"] = _build_nc()
    return _cache["nc"]


def _const_inputs():
    if "consts" not in _cache:
        m96 = np.zeros((H * OCT, H * OCT), dtype=np.float32)
        for h in range(H):
            m96[h * OCT : (h + 1) * OCT, h * OCT : (h + 1) * OCT] = 1.0
        _cache["consts"] = {"m96": m96}
    return _cache["consts"]


def kernel(q, k, v, token_mask):
    from concourse.bass_utils import run_bass_kernel_spmd

    q = np.ascontiguousarray(np.asarray(q, dtype=np.float32))
    k = np.ascontiguousarray(np.asarray(k, dtype=np.float32))
    token_mask = np.ascontiguousarray(np.asarray(token_mask, dtype=np.float32))

    g = _gumbel_np()
    consts = _const_inputs()
    q0 = q[:, :, 0, :]  # [B, H, D]

    nc = _get_nc()
    in_maps = [
        {
            "k": np.ascontiguousarray(k[c]),
            "q": np.ascontiguousarray(q0[c]),
            "tm": token_mask[c],
            "g": np.ascontiguousarray(g[c]),
            "m96": consts["m96"],
        }
        for c in range(N_CORES)
    ]
    res = run_bass_kernel_spmd(nc, in_maps, core_ids=list(range(N_CORES)))
    _cache["last_res"] = res
    out = np.stack([res.results[c]["out"] for c in range(N_CORES)], axis=0)
    return out.astype(bool)


# revision 29
# speedup vs baseline: 1.0770x; 1.0070x over previous
"""Trainium2 Bass kernel for nn_DistributionSampler.

Reference computation (per (batch b, head h)):
  scores  = (K[b,h] @ q[b,h,0,:]) / 8                       # [S]
  attn    = softmax-ish: exp(scores - max) * mask, + eps/S, / (sum + eps)
  top-k   = top-1024 of log(attn) + gumbel(key 42)          # fixed noise
  out     = scatter True at (idx + 1), True at 0, drop OOB  # -> bool [S]

Key identities used by this kernel:
  * The per-head normalizer (max subtraction and sum division) only shifts
    log(attn) by a per-head constant, so the top-k SET is unchanged if we
    use keys = log(exp(dot/8) * mask + eps/S) + gumbel  directly (no max,
    no sum). exp(dot/8) <= e^6-ish, no overflow.
  * The scatter is a shift-by-one: out[j] = (j == 0) | topk_mask[j - 1].
  * Top-k membership == keys > t where t is found by bisection on the
    count of keys above the threshold (exact once the bisection interval
    no longer contains any key; verified bit-exact vs the reference).

Sharding: pure data parallel, core c <- batch c (B=8, 8 cores), all 12
heads per core. No collectives.
"""

import numpy as np

B, H, S, D = 8, 12, 8192, 64
NS = 1024                       # num sampled
EPS = 1e-6
EPS_S = float(np.float32(EPS / S))
N_CORES = 8
P = 128                         # SBUF partitions
A = S // P                      # 64 s-rows per partition in score layout
OCT = 8                         # partitions per head in bisect layout
SB = S // OCT                   # 1024 keys per bisect partition
LO0, HI0 = 1.5, 3.5             # bisection bracket (holds w/ huge margin)
N_ITERS = 22

_cache = {}


def _gumbel_np():
    if "g" not in _cache:
        import jax
        import jax.numpy as jnp

        g = jax.random.gumbel(jax.random.key(42), (B, H, S), dtype=jnp.float32)
        _cache["g"] = np.asarray(g)
    return _cache["g"]


def _build_nc():
    import concourse.bacc as bacc
    import concourse.mybir as mybir
    from concourse.tile import TileContext

    f32 = mybir.dt.float32
    u8 = mybir.dt.uint8
    Alu = mybir.AluOpType
    Act = mybir.ActivationFunctionType

    nc = bacc.Bacc("TRN2", target_bir_lowering=False)

    k_h = nc.dram_tensor("k", [H, S, D], f32, kind="ExternalInput")
    q_h = nc.dram_tensor("q", [H, D], f32, kind="ExternalInput")
    tm_h = nc.dram_tensor("tm", [S], f32, kind="ExternalInput")
    g_h = nc.dram_tensor("g", [H, S], f32, kind="ExternalInput")
    m96_h = nc.dram_tensor("m96", [H * OCT, H * OCT], f32, kind="ExternalInput")
    out_h = nc.dram_tensor("out", [H, S], u8, kind="ExternalOutput")

    NP96 = H * OCT  # 96

    with (
        TileContext(nc) as tc,
        tc.tile_pool(name="kpool", bufs=4) as kpool,
        tc.tile_pool(name="ppool", bufs=2) as ppool,
        tc.tile_pool(name="spool", bufs=3) as spool,
        tc.tile_pool(name="persist", bufs=1) as persist,
        tc.tile_pool(name="psum", bufs=1, space="PSUM") as psum,
    ):
        # ---- persistent small tensors -------------------------------
        mask_sb = persist.tile([P, A], f32, name="mask_sb")
        nc.sync.dma_start(out=mask_sb, in_=tm_h[:].rearrange("(p a) -> p a", p=P))

        m96_sb = persist.tile([NP96, NP96], f32, name="m96_sb")
        nc.sync.dma_start(out=m96_sb, in_=m96_h[:, :])

        keysB = persist.tile([NP96, SB], f32, name="keysB")
        e_all = persist.tile([P, H * A], f32, name="e_all")
        gum_all = persist.tile([P, H * A], f32, name="gum_all")
        # gum_all[p, h*A + a] = g[h, 64p + a]
        nc.sync.dma_start(
            out=gum_all,
            in_=g_h[:, :].rearrange("h (p a) -> p h a", p=P),
        )

        lo_t = persist.tile([NP96, 1], f32, name="lo_t")
        hi_t = persist.tile([NP96, 1], f32, name="hi_t")
        nc.vector.memset(lo_t, LO0)
        nc.vector.memset(hi_t, HI0)

        ones_u8 = persist.tile([H, 1], u8, name="ones_u8")
        nc.vector.memset(ones_u8, 1)
        nc.sync.dma_start(out=out_h[:, 0:1], in_=ones_u8)

        epss_bias = persist.tile([P, 1], f32, name="epss_bias")
        nc.vector.memset(epss_bias, EPS_S)

        # q broadcast to all 128 partitions via a K=1 matmul:
        # qrep_ps[p, (h d)] = q[h, d] for every partition p.
        ones_row = persist.tile([1, P], f32, name="ones_row")
        nc.vector.memset(ones_row, 1.0)
        q_all = persist.tile([1, H * D], f32, name="q_all")
        nc.sync.dma_start(out=q_all, in_=q_h[:, :])

        # [128, 1024] = 2 PSUM banks; two matmuls so neither output AP
        # crosses a bank boundary (N<=512 fp32 per matmul).
        qrep_ps = psum.tile([P, 1024], f32, name="qrep_ps")
        nc.tensor.matmul(
            qrep_ps[:, 0:512], ones_row, q_all[:, 0:512], start=True, stop=True
        )
        nc.tensor.matmul(
            qrep_ps[:, 512 : H * D],
            ones_row,
            q_all[:, 512 : H * D],
            start=True,
            stop=True,
        )

        # ---- per-head score/keys phase ------------------------------
        for h in range(H):
            k_sb = kpool.tile([P, A * D], f32, name="k_sb", tag="k_sb")
            nc.sync.dma_start(
                out=k_sb, in_=k_h[h, :, :].rearrange("(p a) d -> p (a d)", p=P)
            )

            prod = ppool.tile([P, A * D], f32, name="prod", tag="prod")
            nc.vector.tensor_mul(
                prod.rearrange("p (a d) -> p a d", d=D),
                k_sb.rearrange("p (a d) -> p a d", d=D),
                qrep_ps[:, h * D : (h + 1) * D]
                .rearrange("p (x d) -> p x d", x=1)
                .to_broadcast([P, A, D]),
            )

            scores = spool.tile([P, A], f32, name="scores", tag="scores")
            nc.vector.tensor_reduce(
                scores,
                prod.rearrange("p (a d) -> p a d", d=D),
                axis=mybir.AxisListType.X,
                op=Alu.add,
            )

            nc.scalar.activation(
                e_all[:, h * A : (h + 1) * A], scores, Act.Exp, bias=0.0, scale=0.125
            )

        # ---- batched mask/log/gumbel over all heads ------------------
        em_all = persist.tile([P, H * A], f32, name="em_all")
        nc.vector.tensor_mul(
            em_all.rearrange("p (h a) -> p h a", a=A),
            e_all.rearrange("p (h a) -> p h a", a=A),
            mask_sb.rearrange("p (x a) -> p x a", x=1).to_broadcast([P, H, A]),
        )
        lg_all = persist.tile([P, H * A], f32, name="lg_all")
        nc.scalar.activation(lg_all, em_all, Act.Ln, bias=epss_bias, scale=1.0)
        keys_all = persist.tile([P, H * A], f32, name="keys_all")
        nc.vector.tensor_add(keys_all, lg_all, gum_all)
        for h in range(H):
            # [128, 64] column block -> [8, 1024] (same s-order stream)
            eng = nc.sync if h % 2 == 0 else nc.scalar
            eng.dma_start(
                out=keysB[h * OCT : (h + 1) * OCT, :],
                in_=keys_all[:, h * A : (h + 1) * A],
            )

        # ---- bisection ----------------------------------------------
        trash = persist.tile([NP96, SB], u8, name="trash")
        cntall96p = psum.tile([NP96, 1], f32, name="cntall96p", bufs=2)

        for it in range(N_ITERS):
            mid = spool.tile([NP96, 1], f32, name="mid", tag="mid")
            # mid = 0.5*(lo + hi), replicated per octant partition
            nc.vector.scalar_tensor_tensor(
                out=mid,
                in0=lo_t,
                scalar=1.0,
                in1=hi_t,
                op0=Alu.mult,
                op1=Alu.add,
            )
            nc.vector.tensor_scalar(mid, mid, 0.5, None, Alu.mult)

            cnt96 = spool.tile([NP96, 1], f32, name="cnt96", tag="cnt96")
            # per-partition count of keys > mid, in one DVE pass
            nc.vector.tensor_scalar(
                trash, keysB, mid, 0.0, Alu.is_gt, Alu.add, accum_out=cnt96
            )
            # per-head totals, redistributed to every octant partition:
            # cntall96p = blockdiag(ones 8x8).T @ cnt96
            nc.tensor.matmul(cntall96p, m96_sb, cnt96, start=True, stop=True)

            pred = spool.tile([NP96, 1], mybir.dt.uint32, name="pred", tag="pred")
            nc.vector.tensor_scalar(pred, cntall96p, 1024.0, None, Alu.is_ge)
            predn = spool.tile(
                [NP96, 1], mybir.dt.uint32, name="predn", tag="predn"
            )
            nc.vector.tensor_scalar(predn, cntall96p, 1024.0, None, Alu.is_lt)

            nc.vector.copy_predicated(lo_t, pred, mid)
            nc.vector.copy_predicated(hi_t, predn, mid)

        # ---- final mask + output ------------------------------------
        maskB = persist.tile([NP96, SB], u8, name="maskB")
        nc.vector.tensor_scalar(maskB, keysB, lo_t, None, Alu.is_gt)

        # shifted mask
        for h in range(H):
            eng = nc.sync if h % 2 == 0 else nc.scalar
            eng.dma_start(
                out=out_h[h, 1 : 1 + (OCT - 1) * SB].rearrange(
                    "(o x) -> o x", o=OCT - 1
                ),
                in_=maskB[h * OCT : h * OCT + OCT - 1, :],
            )
            eng.dma_start(
                out=out_h[h : h + 1, 1 + (OCT - 1) * SB : S],
                in_=maskB[h * OCT + OCT - 1 : h * OCT + OCT, 0 : SB - 1],
            )

    nc.compile()
    return nc


def _get_nc():
    if "nc" not in _cache:
        _cache["nc"] = _build_nc()
    return _cache["nc"]


def _const_inputs():
    if "consts" not in _cache:
        m96 = np.zeros((H * OCT, H * OCT), dtype=np.float32)
        for h in range(H):
            m96[h * OCT : (h + 1) * OCT, h * OCT : (h + 1) * OCT] = 1.0
        _cache["consts"] = {"m96": m96}
    return _cache["consts"]


def kernel(q, k, v, token_mask):
    from concourse.bass_utils import run_bass_kernel_spmd

    q = np.ascontiguousarray(np.asarray(q, dtype=np.float32))
    k = np.ascontiguousarray(np.asarray(k, dtype=np.float32))
    token_mask = np.ascontiguousarray(np.asarray(token_mask, dtype=np.float32))

    g = _gumbel_np()
    consts = _const_inputs()
    q0 = q[:, :, 0, :]  # [B, H, D]

    nc = _get_nc()
    in_maps = [
        {
            "k": np.ascontiguousarray(k[c]),
            "q": np.ascontiguousarray(q0[c]),
            "tm": token_mask[c],
            "g": np.ascontiguousarray(g[c]),
            "m96": consts["m96"],
        }
        for c in range(N_CORES)
    ]
    res = run_bass_kernel_spmd(nc, in_maps, core_ids=list(range(N_CORES)))
    _cache["last_res"] = res
    out = np.stack([res.results[c]["out"] for c in range(N_CORES)], axis=0)
    return out.astype(bool)


# revision 30
# speedup vs baseline: 1.0797x; 1.0025x over previous
"""Trainium2 Bass kernel for nn_DistributionSampler.

Reference computation (per (batch b, head h)):
  scores  = (K[b,h] @ q[b,h,0,:]) / 8                       # [S]
  attn    = softmax-ish: exp(scores - max) * mask, + eps/S, / (sum + eps)
  top-k   = top-1024 of log(attn) + gumbel(key 42)          # fixed noise
  out     = scatter True at (idx + 1), True at 0, drop OOB  # -> bool [S]

Key identities used by this kernel:
  * The per-head normalizer (max subtraction and sum division) only shifts
    log(attn) by a per-head constant, so the top-k SET is unchanged if we
    use keys = log(exp(dot/8) * mask + eps/S) + gumbel  directly (no max,
    no sum). exp(dot/8) <= e^6-ish, no overflow.
  * The scatter is a shift-by-one: out[j] = (j == 0) | topk_mask[j - 1].
  * Top-k membership == keys > t where t is found by bisection on the
    count of keys above the threshold (exact once the bisection interval
    no longer contains any key; verified bit-exact vs the reference).

Sharding: pure data parallel, core c <- batch c (B=8, 8 cores), all 12
heads per core. No collectives.
"""

import numpy as np

B, H, S, D = 8, 12, 8192, 64
NS = 1024                       # num sampled
EPS = 1e-6
EPS_S = float(np.float32(EPS / S))
N_CORES = 8
P = 128                         # SBUF partitions
A = S // P                      # 64 s-rows per partition in score layout
OCT = 8                         # partitions per head in bisect layout
SB = S // OCT                   # 1024 keys per bisect partition
LO0, HI0 = 1.5, 3.5             # bisection bracket (holds w/ huge margin)
N_ITERS = 22

_cache = {}


def _gumbel_np():
    if "g" not in _cache:
        import jax
        import jax.numpy as jnp

        g = jax.random.gumbel(jax.random.key(42), (B, H, S), dtype=jnp.float32)
        _cache["g"] = np.asarray(g)
    return _cache["g"]


def _build_nc():
    import concourse.bacc as bacc
    import concourse.mybir as mybir
    from concourse.tile import TileContext

    f32 = mybir.dt.float32
    u8 = mybir.dt.uint8
    Alu = mybir.AluOpType
    Act = mybir.ActivationFunctionType

    nc = bacc.Bacc("TRN2", target_bir_lowering=False)

    k_h = nc.dram_tensor("k", [H, S, D], f32, kind="ExternalInput")
    q_h = nc.dram_tensor("q", [H, D], f32, kind="ExternalInput")
    tm_h = nc.dram_tensor("tm", [S], f32, kind="ExternalInput")
    g_h = nc.dram_tensor("g", [H, S], f32, kind="ExternalInput")
    m96_h = nc.dram_tensor("m96", [H * OCT, H * OCT], f32, kind="ExternalInput")
    out_h = nc.dram_tensor("out", [H, S], u8, kind="ExternalOutput")

    NP96 = H * OCT  # 96

    with (
        TileContext(nc) as tc,
        tc.tile_pool(name="kpool", bufs=4) as kpool,
        tc.tile_pool(name="ppool", bufs=2) as ppool,
        tc.tile_pool(name="spool", bufs=3) as spool,
        tc.tile_pool(name="persist", bufs=1) as persist,
        tc.tile_pool(name="psum", bufs=1, space="PSUM") as psum,
    ):
        # ---- prefetch the first two k tiles before anything else ----
        k_pre = {}
        for h in range(2):
            k_sb = kpool.tile([P, A * D], f32, name=f"k_sb{h}", tag="k_sb")
            nc.sync.dma_start(
                out=k_sb, in_=k_h[h, :, :].rearrange("(p a) d -> p (a d)", p=P)
            )
            k_pre[h] = k_sb

        # q must be early too (the qrep matmul gates the first multiply)
        q_all = persist.tile([1, H * D], f32, name="q_all")
        nc.sync.dma_start(out=q_all, in_=q_h[:, :])

        # ---- persistent small tensors -------------------------------
        mask_sb = persist.tile([P, A], f32, name="mask_sb")
        nc.sync.dma_start(out=mask_sb, in_=tm_h[:].rearrange("(p a) -> p a", p=P))

        m96_sb = persist.tile([NP96, NP96], f32, name="m96_sb")
        nc.sync.dma_start(out=m96_sb, in_=m96_h[:, :])

        keysB = persist.tile([NP96, SB], f32, name="keysB")
        e_all = persist.tile([P, H * A], f32, name="e_all")
        gum_all = persist.tile([P, H * A], f32, name="gum_all")
        # gum_all[p, h*A + a] = g[h, 64p + a]
        nc.sync.dma_start(
            out=gum_all,
            in_=g_h[:, :].rearrange("h (p a) -> p h a", p=P),
        )

        lo_t = persist.tile([NP96, 1], f32, name="lo_t")
        hi_t = persist.tile([NP96, 1], f32, name="hi_t")
        nc.vector.memset(lo_t, LO0)
        nc.vector.memset(hi_t, HI0)

        ones_u8 = persist.tile([H, 1], u8, name="ones_u8")
        nc.vector.memset(ones_u8, 1)

        epss_bias = persist.tile([P, 1], f32, name="epss_bias")
        nc.vector.memset(epss_bias, EPS_S)

        # q broadcast to all 128 partitions via a K=1 matmul:
        # qrep_ps[p, (h d)] = q[h, d] for every partition p.
        ones_row = persist.tile([1, P], f32, name="ones_row")
        nc.vector.memset(ones_row, 1.0)
        # [128, 1024] = 2 PSUM banks; two matmuls so neither output AP
        # crosses a bank boundary (N<=512 fp32 per matmul).
        qrep_ps = psum.tile([P, 1024], f32, name="qrep_ps")
        nc.tensor.matmul(
            qrep_ps[:, 0:512], ones_row, q_all[:, 0:512], start=True, stop=True
        )
        nc.tensor.matmul(
            qrep_ps[:, 512 : H * D],
            ones_row,
            q_all[:, 512 : H * D],
            start=True,
            stop=True,
        )

        # ---- per-head score/keys phase ------------------------------
        for h in range(H):
            if h in k_pre:
                k_sb = k_pre[h]
            else:
                k_sb = kpool.tile([P, A * D], f32, name="k_sb", tag="k_sb")
                nc.sync.dma_start(
                    out=k_sb,
                    in_=k_h[h, :, :].rearrange("(p a) d -> p (a d)", p=P),
                )

            prod = ppool.tile([P, A * D], f32, name="prod", tag="prod")
            nc.vector.tensor_mul(
                prod.rearrange("p (a d) -> p a d", d=D),
                k_sb.rearrange("p (a d) -> p a d", d=D),
                qrep_ps[:, h * D : (h + 1) * D]
                .rearrange("p (x d) -> p x d", x=1)
                .to_broadcast([P, A, D]),
            )

            scores = spool.tile([P, A], f32, name="scores", tag="scores")
            nc.vector.tensor_reduce(
                scores,
                prod.rearrange("p (a d) -> p a d", d=D),
                axis=mybir.AxisListType.X,
                op=Alu.add,
            )

            nc.scalar.activation(
                e_all[:, h * A : (h + 1) * A], scores, Act.Exp, bias=0.0, scale=0.125
            )

        # ---- batched mask/log/gumbel over all heads ------------------
        em_all = persist.tile([P, H * A], f32, name="em_all")
        nc.vector.tensor_mul(
            em_all.rearrange("p (h a) -> p h a", a=A),
            e_all.rearrange("p (h a) -> p h a", a=A),
            mask_sb.rearrange("p (x a) -> p x a", x=1).to_broadcast([P, H, A]),
        )
        lg_all = persist.tile([P, H * A], f32, name="lg_all")
        nc.scalar.activation(lg_all, em_all, Act.Ln, bias=epss_bias, scale=1.0)
        keys_all = persist.tile([P, H * A], f32, name="keys_all")
        nc.vector.tensor_add(keys_all, lg_all, gum_all)
        for h in range(H):
            # [128, 64] column block -> [8, 1024] (same s-order stream)
            eng = nc.sync if h % 2 == 0 else nc.scalar
            eng.dma_start(
                out=keysB[h * OCT : (h + 1) * OCT, :],
                in_=keys_all[:, h * A : (h + 1) * A],
            )

        # ---- bisection ----------------------------------------------
        trash = persist.tile([NP96, SB], u8, name="trash")
        cntall96p = psum.tile([NP96, 1], f32, name="cntall96p", bufs=2)

        for it in range(N_ITERS):
            mid = spool.tile([NP96, 1], f32, name="mid", tag="mid")
            # mid = 0.5*(lo + hi), replicated per octant partition
            nc.vector.scalar_tensor_tensor(
                out=mid,
                in0=lo_t,
                scalar=1.0,
                in1=hi_t,
                op0=Alu.mult,
                op1=Alu.add,
            )
            nc.vector.tensor_scalar(mid, mid, 0.5, None, Alu.mult)

            cnt96 = spool.tile([NP96, 1], f32, name="cnt96", tag="cnt96")
            # per-partition count of keys > mid, in one DVE pass
            nc.vector.tensor_scalar(
                trash, keysB, mid, 0.0, Alu.is_gt, Alu.add, accum_out=cnt96
            )
            # per-head totals, redistributed to every octant partition:
            # cntall96p = blockdiag(ones 8x8).T @ cnt96
            nc.tensor.matmul(cntall96p, m96_sb, cnt96, start=True, stop=True)

            pred = spool.tile([NP96, 1], mybir.dt.uint32, name="pred", tag="pred")
            nc.vector.tensor_scalar(pred, cntall96p, 1024.0, None, Alu.is_ge)
            predn = spool.tile(
                [NP96, 1], mybir.dt.uint32, name="predn", tag="predn"
            )
            nc.vector.tensor_scalar(predn, cntall96p, 1024.0, None, Alu.is_lt)

            nc.vector.copy_predicated(lo_t, pred, mid)
            nc.vector.copy_predicated(hi_t, predn, mid)

        # ---- final mask + output ------------------------------------
        maskB = persist.tile([NP96, SB], u8, name="maskB")
        nc.vector.tensor_scalar(maskB, keysB, lo_t, None, Alu.is_gt)

        # shifted mask: for h < 11 write all 8192 flags at offset 1; the
        # final byte lands on out[h+1, 0] (the next head's class token,
        # overwritten below).  Head 11 is split to stay in bounds.
        flat = out_h[:, :].rearrange("h s -> (h s)")
        for h in range(H):
            eng = nc.sync if h % 2 == 0 else nc.scalar
            if h < H - 1:
                eng.dma_start(
                    out=flat[h * S + 1 : (h + 1) * S + 1].rearrange(
                        "(o x) -> o x", o=OCT
                    ),
                    in_=maskB[h * OCT : (h + 1) * OCT, :],
                )
            else:
                eng.dma_start(
                    out=out_h[h, 1 : 1 + (OCT - 1) * SB].rearrange(
                        "(o x) -> o x", o=OCT - 1
                    ),
                    in_=maskB[h * OCT : h * OCT + OCT - 1, :],
                )
                eng.dma_start(
                    out=out_h[h : h + 1, 1 + (OCT - 1) * SB : S],
                    in_=maskB[h * OCT + OCT - 1 : h * OCT + OCT, 0 : SB - 1],
                )
        # class tokens last: overwrite the spill bytes with True
        nc.scalar.dma_start(out=out_h[:, 0:1], in_=ones_u8)

    nc.compile()
    return nc


def _get_nc():
    if "nc" not in _cache:
        _cache["nc"] = _build_nc()
    return _cache["nc"]


def _const_inputs():
    if "consts" not in _cache:
        m96 = np.zeros((H * OCT, H * OCT), dtype=np.float32)
        for h in range(H):
            m96[h * OCT : (h + 1) * OCT, h * OCT : (h + 1) * OCT] = 1.0
        _cache["consts"] = {"m96": m96}
    return _cache["consts"]


def kernel(q, k, v, token_mask):
    from concourse.bass_utils import run_bass_kernel_spmd

    q = np.ascontiguousarray(np.asarray(q, dtype=np.float32))
    k = np.ascontiguousarray(np.asarray(k, dtype=np.float32))
    token_mask = np.ascontiguousarray(np.asarray(token_mask, dtype=np.float32))

    g = _gumbel_np()
    consts = _const_inputs()
    q0 = q[:, :, 0, :]  # [B, H, D]

    nc = _get_nc()
    in_maps = [
        {
            "k": np.ascontiguousarray(k[c]),
            "q": np.ascontiguousarray(q0[c]),
            "tm": token_mask[c],
            "g": np.ascontiguousarray(g[c]),
            "m96": consts["m96"],
        }
        for c in range(N_CORES)
    ]
    res = run_bass_kernel_spmd(nc, in_maps, core_ids=list(range(N_CORES)))
    _cache["last_res"] = res
    out = np.stack([res.results[c]["out"] for c in range(N_CORES)], axis=0)
    return out.astype(bool)


# revision 31
# speedup vs baseline: 1.1218x; 1.0391x over previous
"""Trainium2 Bass kernel for nn_DistributionSampler.

Reference computation (per (batch b, head h)):
  scores  = (K[b,h] @ q[b,h,0,:]) / 8                       # [S]
  attn    = softmax-ish: exp(scores - max) * mask, + eps/S, / (sum + eps)
  top-k   = top-1024 of log(attn) + gumbel(key 42)          # fixed noise
  out     = scatter True at (idx + 1), True at 0, drop OOB  # -> bool [S]

Key identities used by this kernel:
  * The per-head normalizer (max subtraction and sum division) only shifts
    log(attn) by a per-head constant, so the top-k SET is unchanged if we
    use keys = log(exp(dot/8) * mask + eps/S) + gumbel  directly (no max,
    no sum). exp(dot/8) <= e^6-ish, no overflow.
  * The scatter is a shift-by-one: out[j] = (j == 0) | topk_mask[j - 1].
  * Top-k membership == keys > t where t is found by bisection on the
    count of keys above the threshold (exact once the bisection interval
    no longer contains any key; verified bit-exact vs the reference).

Sharding: pure data parallel, core c <- batch c (B=8, 8 cores), all 12
heads per core. No collectives.
"""

import numpy as np

B, H, S, D = 8, 12, 8192, 64
NS = 1024                       # num sampled
EPS = 1e-6
EPS_S = float(np.float32(EPS / S))
N_CORES = 8
P = 128                         # SBUF partitions
A = S // P                      # 64 s-rows per partition in score layout
OCT = 8                         # partitions per head in bisect layout
SB = S // OCT                   # 1024 keys per bisect partition
LO0, HI0 = 1.5, 3.5             # bisection bracket (holds w/ huge margin)
N_ITERS = 22

_cache = {}


def _gumbel_np():
    if "g" not in _cache:
        import jax
        import jax.numpy as jnp

        g = jax.random.gumbel(jax.random.key(42), (B, H, S), dtype=jnp.float32)
        _cache["g"] = np.asarray(g)
    return _cache["g"]


def _build_nc():
    import concourse.bacc as bacc
    import concourse.mybir as mybir
    from concourse.tile import TileContext

    f32 = mybir.dt.float32
    u8 = mybir.dt.uint8
    Alu = mybir.AluOpType
    Act = mybir.ActivationFunctionType

    nc = bacc.Bacc("TRN2", target_bir_lowering=False)

    k_h = nc.dram_tensor("k", [H, S, D], f32, kind="ExternalInput")
    q_h = nc.dram_tensor("q", [H, D], f32, kind="ExternalInput")
    tm_h = nc.dram_tensor("tm", [S], f32, kind="ExternalInput")
    g_h = nc.dram_tensor("g", [H, S], f32, kind="ExternalInput")
    m96_h = nc.dram_tensor("m96", [H * OCT, H * OCT], f32, kind="ExternalInput")
    out_h = nc.dram_tensor("out", [H, S], u8, kind="ExternalOutput")

    NP96 = H * OCT  # 96

    with (
        TileContext(nc) as tc,
        tc.tile_pool(name="kpool", bufs=4) as kpool,
        tc.tile_pool(name="ppool", bufs=2) as ppool,
        tc.tile_pool(name="spool", bufs=3) as spool,
        tc.tile_pool(name="persist", bufs=1) as persist,
        tc.tile_pool(name="psum", bufs=1, space="PSUM") as psum,
    ):
        # q absolutely first (the qrep matmul gates the first multiply),
        # then the first two k tiles, then everything else.
        q_all = persist.tile([1, H * D], f32, name="q_all")
        nc.sync.dma_start(out=q_all, in_=q_h[:, :])

        k_pre = {}
        for h in range(2):
            k_sb = kpool.tile([P, A * D], f32, name=f"k_sb{h}", tag="k_sb")
            nc.sync.dma_start(
                out=k_sb, in_=k_h[h, :, :].rearrange("(p a) d -> p (a d)", p=P)
            )
            k_pre[h] = k_sb

        # ---- persistent small tensors -------------------------------
        mask_sb = persist.tile([P, A], f32, name="mask_sb")
        nc.sync.dma_start(out=mask_sb, in_=tm_h[:].rearrange("(p a) -> p a", p=P))

        m96_sb = persist.tile([NP96, NP96], f32, name="m96_sb")
        nc.sync.dma_start(out=m96_sb, in_=m96_h[:, :])

        keysB = persist.tile([NP96, SB], f32, name="keysB")
        e_all = persist.tile([P, H * A], f32, name="e_all")
        gum_all = persist.tile([P, H * A], f32, name="gum_all")
        # gum_all[p, h*A + a] = g[h, 64p + a]
        nc.sync.dma_start(
            out=gum_all,
            in_=g_h[:, :].rearrange("h (p a) -> p h a", p=P),
        )

        lo_t = persist.tile([NP96, 1], f32, name="lo_t")
        hi_t = persist.tile([NP96, 1], f32, name="hi_t")
        nc.vector.memset(lo_t, LO0)
        nc.vector.memset(hi_t, HI0)

        ones_u8 = persist.tile([H, 1], u8, name="ones_u8")
        nc.vector.memset(ones_u8, 1)

        epss_bias = persist.tile([P, 1], f32, name="epss_bias")
        nc.vector.memset(epss_bias, EPS_S)

        # q broadcast to all 128 partitions via a K=1 matmul:
        # qrep_ps[p, (h d)] = q[h, d] for every partition p.
        ones_row = persist.tile([1, P], f32, name="ones_row")
        nc.vector.memset(ones_row, 1.0)
        # [128, 1024] = 2 PSUM banks; two matmuls so neither output AP
        # crosses a bank boundary (N<=512 fp32 per matmul).
        qrep_ps = psum.tile([P, 1024], f32, name="qrep_ps")
        nc.tensor.matmul(
            qrep_ps[:, 0:512], ones_row, q_all[:, 0:512], start=True, stop=True
        )
        nc.tensor.matmul(
            qrep_ps[:, 512 : H * D],
            ones_row,
            q_all[:, 512 : H * D],
            start=True,
            stop=True,
        )

        # ---- per-head score/keys phase ------------------------------
        for h in range(H):
            if h in k_pre:
                k_sb = k_pre[h]
            else:
                k_sb = kpool.tile([P, A * D], f32, name="k_sb", tag="k_sb")
                nc.sync.dma_start(
                    out=k_sb,
                    in_=k_h[h, :, :].rearrange("(p a) d -> p (a d)", p=P),
                )

            prod = ppool.tile([P, A * D], f32, name="prod", tag="prod")
            nc.vector.tensor_mul(
                prod.rearrange("p (a d) -> p a d", d=D),
                k_sb.rearrange("p (a d) -> p a d", d=D),
                qrep_ps[:, h * D : (h + 1) * D]
                .rearrange("p (x d) -> p x d", x=1)
                .to_broadcast([P, A, D]),
            )

            scores = spool.tile([P, A], f32, name="scores", tag="scores")
            nc.vector.tensor_reduce(
                scores,
                prod.rearrange("p (a d) -> p a d", d=D),
                axis=mybir.AxisListType.X,
                op=Alu.add,
            )

            nc.scalar.activation(
                e_all[:, h * A : (h + 1) * A], scores, Act.Exp, bias=0.0, scale=0.125
            )

        # ---- batched mask/log/gumbel over all heads ------------------
        em_all = persist.tile([P, H * A], f32, name="em_all")
        nc.vector.tensor_mul(
            em_all.rearrange("p (h a) -> p h a", a=A),
            e_all.rearrange("p (h a) -> p h a", a=A),
            mask_sb.rearrange("p (x a) -> p x a", x=1).to_broadcast([P, H, A]),
        )
        lg_all = persist.tile([P, H * A], f32, name="lg_all")
        nc.scalar.activation(lg_all, em_all, Act.Ln, bias=epss_bias, scale=1.0)
        keys_all = persist.tile([P, H * A], f32, name="keys_all")
        nc.vector.tensor_add(keys_all, lg_all, gum_all)
        for h in range(H):
            # [128, 64] column block -> [8, 1024] (same s-order stream)
            eng = nc.sync if h % 2 == 0 else nc.scalar
            eng.dma_start(
                out=keysB[h * OCT : (h + 1) * OCT, :],
                in_=keys_all[:, h * A : (h + 1) * A],
            )

        # ---- bisection ----------------------------------------------
        trash = persist.tile([NP96, SB], u8, name="trash")
        cntall96p = psum.tile([NP96, 1], f32, name="cntall96p", bufs=2)

        for it in range(N_ITERS):
            mid = spool.tile([NP96, 1], f32, name="mid", tag="mid")
            # mid = 0.5*(lo + hi), replicated per octant partition
            nc.vector.scalar_tensor_tensor(
                out=mid,
                in0=lo_t,
                scalar=1.0,
                in1=hi_t,
                op0=Alu.mult,
                op1=Alu.add,
            )
            nc.vector.tensor_scalar(mid, mid, 0.5, None, Alu.mult)

            cnt96 = spool.tile([NP96, 1], f32, name="cnt96", tag="cnt96")
            # per-partition count of keys > mid, in one DVE pass
            nc.vector.tensor_scalar(
                trash, keysB, mid, 0.0, Alu.is_gt, Alu.add, accum_out=cnt96
            )
            # per-head totals, redistributed to every octant partition:
            # cntall96p = blockdiag(ones 8x8).T @ cnt96
            nc.tensor.matmul(cntall96p, m96_sb, cnt96, start=True, stop=True)

            pred = spool.tile([NP96, 1], mybir.dt.uint32, name="pred", tag="pred")
            nc.vector.tensor_scalar(pred, cntall96p, 1024.0, None, Alu.is_ge)
            predn = spool.tile(
                [NP96, 1], mybir.dt.uint32, name="predn", tag="predn"
            )
            nc.vector.tensor_scalar(predn, cntall96p, 1024.0, None, Alu.is_lt)

            nc.vector.copy_predicated(lo_t, pred, mid)
            nc.vector.copy_predicated(hi_t, predn, mid)

        # ---- final mask + output ------------------------------------
        maskB = persist.tile([NP96, SB], u8, name="maskB")
        nc.vector.tensor_scalar(maskB, keysB, lo_t, None, Alu.is_gt)

        # shifted mask: for h < 11 write all 8192 flags at offset 1; the
        # final byte lands on out[h+1, 0] (the next head's class token,
        # overwritten below).  Head 11 is split to stay in bounds.
        flat = out_h[:, :].rearrange("h s -> (h s)")
        for h in range(H):
            eng = nc.sync if h % 2 == 0 else nc.scalar
            if h < H - 1:
                eng.dma_start(
                    out=flat[h * S + 1 : (h + 1) * S + 1].rearrange(
                        "(o x) -> o x", o=OCT
                    ),
                    in_=maskB[h * OCT : (h + 1) * OCT, :],
                )
            else:
                eng.dma_start(
                    out=out_h[h, 1 : 1 + (OCT - 1) * SB].rearrange(
                        "(o x) -> o x", o=OCT - 1
                    ),
                    in_=maskB[h * OCT : h * OCT + OCT - 1, :],
                )
                eng.dma_start(
                    out=out_h[h : h + 1, 1 + (OCT - 1) * SB : S],
                    in_=maskB[h * OCT + OCT - 1 : h * OCT + OCT, 0 : SB - 1],
                )
        # class tokens last: overwrite the spill bytes with True
        nc.scalar.dma_start(out=out_h[:, 0:1], in_=ones_u8)

    nc.compile()
    return nc


def _get_nc():
    if "nc" not in _cache:
        _cache["nc"] = _build_nc()
    return _cache["nc"]


def _const_inputs():
    if "consts" not in _cache:
        m96 = np.zeros((H * OCT, H * OCT), dtype=np.float32)
        for h in range(H):
            m96[h * OCT : (h + 1) * OCT, h * OCT : (h + 1) * OCT] = 1.0
        _cache["consts"] = {"m96": m96}
    return _cache["consts"]


def kernel(q, k, v, token_mask):
    from concourse.bass_utils import run_bass_kernel_spmd

    q = np.ascontiguousarray(np.asarray(q, dtype=np.float32))
    k = np.ascontiguousarray(np.asarray(k, dtype=np.float32))
    token_mask = np.ascontiguousarray(np.asarray(token_mask, dtype=np.float32))

    g = _gumbel_np()
    consts = _const_inputs()
    q0 = q[:, :, 0, :]  # [B, H, D]

    nc = _get_nc()
    in_maps = [
        {
            "k": np.ascontiguousarray(k[c]),
            "q": np.ascontiguousarray(q0[c]),
            "tm": token_mask[c],
            "g": np.ascontiguousarray(g[c]),
            "m96": consts["m96"],
        }
        for c in range(N_CORES)
    ]
    res = run_bass_kernel_spmd(nc, in_maps, core_ids=list(range(N_CORES)))
    _cache["last_res"] = res
    out = np.stack([res.results[c]["out"] for c in range(N_CORES)], axis=0)
    return out.astype(bool)


# revision 32
# speedup vs baseline: 1.1489x; 1.0241x over previous
"""Trainium2 Bass kernel for nn_DistributionSampler.

Reference computation (per (batch b, head h)):
  scores  = (K[b,h] @ q[b,h,0,:]) / 8                       # [S]
  attn    = softmax-ish: exp(scores - max) * mask, + eps/S, / (sum + eps)
  top-k   = top-1024 of log(attn) + gumbel(key 42)          # fixed noise
  out     = scatter True at (idx + 1), True at 0, drop OOB  # -> bool [S]

Key identities used by this kernel:
  * The per-head normalizer (max subtraction and sum division) only shifts
    log(attn) by a per-head constant, so the top-k SET is unchanged if we
    use keys = log(exp(dot/8) * mask + eps/S) + gumbel  directly (no max,
    no sum). exp(dot/8) <= e^6-ish, no overflow.
  * The scatter is a shift-by-one: out[j] = (j == 0) | topk_mask[j - 1].
  * Top-k membership == keys > t where t is found by bisection on the
    count of keys above the threshold (exact once the bisection interval
    no longer contains any key; verified bit-exact vs the reference).

Sharding: pure data parallel, core c <- batch c (B=8, 8 cores), all 12
heads per core. No collectives.
"""

import numpy as np

B, H, S, D = 8, 12, 8192, 64
NS = 1024                       # num sampled
EPS = 1e-6
EPS_S = float(np.float32(EPS / S))
N_CORES = 8
P = 128                         # SBUF partitions
A = S // P                      # 64 s-rows per partition in score layout
OCT = 8                         # partitions per head in bisect layout
SB = S // OCT                   # 1024 keys per bisect partition
LO0, HI0 = 1.5, 3.5             # bisection bracket (holds w/ huge margin)
N_ITERS = 20

_cache = {}


def _gumbel_np():
    if "g" not in _cache:
        import jax
        import jax.numpy as jnp

        g = jax.random.gumbel(jax.random.key(42), (B, H, S), dtype=jnp.float32)
        _cache["g"] = np.asarray(g)
    return _cache["g"]


def _build_nc():
    import concourse.bacc as bacc
    import concourse.mybir as mybir
    from concourse.tile import TileContext

    f32 = mybir.dt.float32
    u8 = mybir.dt.uint8
    Alu = mybir.AluOpType
    Act = mybir.ActivationFunctionType

    nc = bacc.Bacc("TRN2", target_bir_lowering=False)

    k_h = nc.dram_tensor("k", [H, S, D], f32, kind="ExternalInput")
    q_h = nc.dram_tensor("q", [H, D], f32, kind="ExternalInput")
    tm_h = nc.dram_tensor("tm", [S], f32, kind="ExternalInput")
    g_h = nc.dram_tensor("g", [H, S], f32, kind="ExternalInput")
    m96_h = nc.dram_tensor("m96", [H * OCT, H * OCT], f32, kind="ExternalInput")
    out_h = nc.dram_tensor("out", [H, S], u8, kind="ExternalOutput")

    NP96 = H * OCT  # 96

    with (
        TileContext(nc) as tc,
        tc.tile_pool(name="kpool", bufs=4) as kpool,
        tc.tile_pool(name="ppool", bufs=2) as ppool,
        tc.tile_pool(name="spool", bufs=3) as spool,
        tc.tile_pool(name="persist", bufs=1) as persist,
        tc.tile_pool(name="psum", bufs=1, space="PSUM") as psum,
    ):
        # q absolutely first (the qrep matmul gates the first multiply),
        # then the first two k tiles, then everything else.
        q_all = persist.tile([1, H * D], f32, name="q_all")
        nc.sync.dma_start(out=q_all, in_=q_h[:, :])

        k_pre = {}
        for h in range(2):
            k_sb = kpool.tile([P, A * D], f32, name=f"k_sb{h}", tag="k_sb")
            nc.sync.dma_start(
                out=k_sb, in_=k_h[h, :, :].rearrange("(p a) d -> p (a d)", p=P)
            )
            k_pre[h] = k_sb

        # ---- persistent small tensors -------------------------------
        mask_sb = persist.tile([P, A], f32, name="mask_sb")
        nc.sync.dma_start(out=mask_sb, in_=tm_h[:].rearrange("(p a) -> p a", p=P))

        m96_sb = persist.tile([NP96, NP96], f32, name="m96_sb")
        nc.sync.dma_start(out=m96_sb, in_=m96_h[:, :])

        keysB = persist.tile([NP96, SB], f32, name="keysB")
        e_all = persist.tile([P, H * A], f32, name="e_all")
        gum_all = persist.tile([P, H * A], f32, name="gum_all")
        # gum_all[p, h*A + a] = g[h, 64p + a]
        nc.scalar.dma_start(
            out=gum_all,
            in_=g_h[:, :].rearrange("h (p a) -> p h a", p=P),
        )

        lo_t = persist.tile([NP96, 1], f32, name="lo_t")
        hi_t = persist.tile([NP96, 1], f32, name="hi_t")
        nc.vector.memset(lo_t, LO0)
        nc.vector.memset(hi_t, HI0)

        ones_u8 = persist.tile([H, 1], u8, name="ones_u8")
        nc.vector.memset(ones_u8, 1)

        epss_bias = persist.tile([P, 1], f32, name="epss_bias")
        nc.vector.memset(epss_bias, EPS_S)

        # q broadcast to all 128 partitions via a K=1 matmul:
        # qrep_ps[p, (h d)] = q[h, d] for every partition p.
        ones_row = persist.tile([1, P], f32, name="ones_row")
        nc.vector.memset(ones_row, 1.0)
        # [128, 1024] = 2 PSUM banks; two matmuls so neither output AP
        # crosses a bank boundary (N<=512 fp32 per matmul).
        qrep_ps = psum.tile([P, 1024], f32, name="qrep_ps")
        nc.tensor.matmul(
            qrep_ps[:, 0:512], ones_row, q_all[:, 0:512], start=True, stop=True
        )
        nc.tensor.matmul(
            qrep_ps[:, 512 : H * D],
            ones_row,
            q_all[:, 512 : H * D],
            start=True,
            stop=True,
        )

        # ---- per-head score/keys phase ------------------------------
        for h in range(H):
            if h in k_pre:
                k_sb = k_pre[h]
            else:
                k_sb = kpool.tile([P, A * D], f32, name="k_sb", tag="k_sb")
                nc.sync.dma_start(
                    out=k_sb,
                    in_=k_h[h, :, :].rearrange("(p a) d -> p (a d)", p=P),
                )

            prod = ppool.tile([P, A * D], f32, name="prod", tag="prod")
            nc.vector.tensor_mul(
                prod.rearrange("p (a d) -> p a d", d=D),
                k_sb.rearrange("p (a d) -> p a d", d=D),
                qrep_ps[:, h * D : (h + 1) * D]
                .rearrange("p (x d) -> p x d", x=1)
                .to_broadcast([P, A, D]),
            )

            scores = spool.tile([P, A], f32, name="scores", tag="scores")
            nc.vector.tensor_reduce(
                scores,
                prod.rearrange("p (a d) -> p a d", d=D),
                axis=mybir.AxisListType.X,
                op=Alu.add,
            )

            nc.scalar.activation(
                e_all[:, h * A : (h + 1) * A], scores, Act.Exp, bias=0.0, scale=0.125
            )

        # ---- batched mask/log/gumbel over all heads ------------------
        em_all = persist.tile([P, H * A], f32, name="em_all")
        nc.vector.tensor_mul(
            em_all.rearrange("p (h a) -> p h a", a=A),
            e_all.rearrange("p (h a) -> p h a", a=A),
            mask_sb.rearrange("p (x a) -> p x a", x=1).to_broadcast([P, H, A]),
        )
        lg_all = persist.tile([P, H * A], f32, name="lg_all")
        nc.scalar.activation(lg_all, em_all, Act.Ln, bias=epss_bias, scale=1.0)
        keys_all = persist.tile([P, H * A], f32, name="keys_all")
        nc.vector.tensor_add(keys_all, lg_all, gum_all)
        for h in range(H):
            # [128, 64] column block -> [8, 1024] (same s-order stream)
            eng = nc.sync if h % 2 == 0 else nc.scalar
            eng.dma_start(
                out=keysB[h * OCT : (h + 1) * OCT, :],
                in_=keys_all[:, h * A : (h + 1) * A],
            )

        # ---- bisection ----------------------------------------------
        trash = persist.tile([NP96, SB], u8, name="trash")
        cntall96p = psum.tile([NP96, 1], f32, name="cntall96p", bufs=2)

        for it in range(N_ITERS):
            mid = spool.tile([NP96, 1], f32, name="mid", tag="mid")
            # mid = 0.5*(lo + hi), replicated per octant partition
            nc.vector.scalar_tensor_tensor(
                out=mid,
                in0=lo_t,
                scalar=1.0,
                in1=hi_t,
                op0=Alu.mult,
                op1=Alu.add,
            )
            nc.vector.tensor_scalar(mid, mid, 0.5, None, Alu.mult)

            cnt96 = spool.tile([NP96, 1], f32, name="cnt96", tag="cnt96")
            # per-partition count of keys > mid, in one DVE pass
            nc.vector.tensor_scalar(
                trash, keysB, mid, 0.0, Alu.is_gt, Alu.add, accum_out=cnt96
            )
            # per-head totals, redistributed to every octant partition:
            # cntall96p = blockdiag(ones 8x8).T @ cnt96
            nc.tensor.matmul(cntall96p, m96_sb, cnt96, start=True, stop=True)

            pred = spool.tile([NP96, 1], mybir.dt.uint32, name="pred", tag="pred")
            nc.vector.tensor_scalar(pred, cntall96p, 1024.0, None, Alu.is_ge)
            predn = spool.tile(
                [NP96, 1], mybir.dt.uint32, name="predn", tag="predn"
            )
            nc.vector.tensor_scalar(predn, cntall96p, 1024.0, None, Alu.is_lt)

            nc.vector.copy_predicated(lo_t, pred, mid)
            nc.vector.copy_predicated(hi_t, predn, mid)

        # ---- final mask + output ------------------------------------
        maskB = persist.tile([NP96, SB], u8, name="maskB")
        nc.vector.tensor_scalar(maskB, keysB, lo_t, None, Alu.is_gt)

        # shifted mask: for h < 11 write all 8192 flags at offset 1; the
        # final byte lands on out[h+1, 0] (the next head's class token,
        # overwritten below).  Head 11 is split to stay in bounds.
        flat = out_h[:, :].rearrange("h s -> (h s)")
        for h in range(H):
            eng = nc.sync if h % 2 == 0 else nc.scalar
            if h < H - 1:
                eng.dma_start(
                    out=flat[h * S + 1 : (h + 1) * S + 1].rearrange(
                        "(o x) -> o x", o=OCT
                    ),
                    in_=maskB[h * OCT : (h + 1) * OCT, :],
                )
            else:
                eng.dma_start(
                    out=out_h[h, 1 : 1 + (OCT - 1) * SB].rearrange(
                        "(o x) -> o x", o=OCT - 1
                    ),
                    in_=maskB[h * OCT : h * OCT + OCT - 1, :],
                )
                eng.dma_start(
                    out=out_h[h : h + 1, 1 + (OCT - 1) * SB : S],
                    in_=maskB[h * OCT + OCT - 1 : h * OCT + OCT, 0 : SB - 1],
                )
        # class tokens last: overwrite the spill bytes with True
        nc.scalar.dma_start(out=out_h[:, 0:1], in_=ones_u8)

    nc.compile()
    return nc


def _get_nc():
    if "nc" not in _cache:
        _cache["nc"] = _build_nc()
    return _cache["nc"]


def _const_inputs():
    if "consts" not in _cache:
        m96 = np.zeros((H * OCT, H * OCT), dtype=np.float32)
        for h in range(H):
            m96[h * OCT : (h + 1) * OCT, h * OCT : (h + 1) * OCT] = 1.0
        _cache["consts"] = {"m96": m96}
    return _cache["consts"]


def kernel(q, k, v, token_mask):
    from concourse.bass_utils import run_bass_kernel_spmd

    q = np.ascontiguousarray(np.asarray(q, dtype=np.float32))
    k = np.ascontiguousarray(np.asarray(k, dtype=np.float32))
    token_mask = np.ascontiguousarray(np.asarray(token_mask, dtype=np.float32))

    g = _gumbel_np()
    consts = _const_inputs()
    q0 = q[:, :, 0, :]  # [B, H, D]

    nc = _get_nc()
    in_maps = [
        {
            "k": np.ascontiguousarray(k[c]),
            "q": np.ascontiguousarray(q0[c]),
            "tm": token_mask[c],
            "g": np.ascontiguousarray(g[c]),
            "m96": consts["m96"],
        }
        for c in range(N_CORES)
    ]
    res = run_bass_kernel_spmd(nc, in_maps, core_ids=list(range(N_CORES)))
    _cache["last_res"] = res
    out = np.stack([res.results[c]["out"] for c in range(N_CORES)], axis=0)
    return out.astype(bool)
